# revision 32
# baseline (speedup 1.0000x reference)
"""Trainium2 Bass kernel for BinaryTreeLatentVariable inside algorithm.

Math (per level d, bottom-up over a complete binary tree in heap order):
    new[pp, n] = p[pp, n] + logsumexp_{i,j}( trans[pp, i, j] + l[i, n] + r[j, n] )

Factorization (s[n] = l[0, n] + r[0, n]):
    new[pp, n] = p[pp, n] + s[n] + log( sum_{ij} expT[ij, pp] * V[ij, n] )
    V[ij, n]   = exp( lnorm[i, n] + rnorm[j, n] ),  xnorm[i] = x[i] - x[0]
    expT       = exp(trans) permuted to [(lL,lc),(rL,rc)] x [(pL,pc)]

Representation: per-level buffers store u[s, n] = t[s, n] + sw_norm[s, n]
(t = ln of the contraction; sw_norm = emission with col s -> W_s - W_0
except col 0 which keeps the absolute state-0 emission).  u differs from
the true in_w by a per-node constant shift (the dropped carry), so the
normalization lnorm[i] = in_w[i] - in_w[0] equals u[i] - u[0]: the select
matrices carry the subtraction directly (-1 entries on the state-0 rows)
and no separate normalization matmul is needed.  Leaves store plain
absolute emissions (same property with zero shift).  The per-node carry
(row 0 / row 20 of each buffer) is folded into a per-tree fp32 z
accumulator; the root output adds the total back via a rank-1 matmul.

Buffers are DEINTERLEAVED: sibling pairs share a column (left child in
partitions 0..19, right child in partitions 32..51; 20..31 are zeroed
once — engine accesses must not cross a 32-partition boundary).

Per level tile (nodes on the free axis):
    - 4x select matmul (K=40, bf16 0/+-1 matrix): args = lnorm_i + rnorm_j,
      chunks packed into paired PSUM banks so one ACT exp covers 2 (big
      tiles) or all 4 (nt <= 256) chunks
    - ACT exp (PSUM -> SBUF bf16), 4x contraction matmul with expT
    - ACT ln of the accumulated sums, then DVE adds u = ln + p with a
      strided deinterleave into the next level's buffer
    - DVE carries the fp32 z-chain (off the critical path)

Phase 1: emission sw = W^T @ hT + b on PE; h is cast to bf16 host-side
(halving HBM traffic) and laid out level-major (leaves first) so the
deepest level overlaps the tail of the h DMA.  Leaf columns use plain W
(absolute representation); internal columns use host-normalized Wn.

Dummy PE matmuls (reading constants, writing a dedicated scratch PSUM
bank) are woven between real work to keep the PE HAM clock gate at full
speed (2.4 GHz) through DMA- and ACT-bound stretches.

Sharding: 8 trees per core across 8 cores (no cross-core communication).
"""

import ml_dtypes
import numpy as np

import concourse.bacc as bacc
import concourse.bass as bass
from concourse import mybir, tile
from concourse.bass_utils import run_bass_kernel_spmd

F32 = mybir.dt.float32
BF16 = mybir.dt.bfloat16
FP8 = mybir.dt.float8e4
NP_BF16 = ml_dtypes.bfloat16
NP_FP8 = ml_dtypes.float8_e4m3fn

B = 64
N_NODES = 1023
D = 512
L = 5
C = 4
LC = L * C          # 20
IJ = 400            # 20 * 20
NCORES = 8
TPC = B // NCORES   # trees per core = 8
DEPTH = 9           # leaves are level 9; internal levels 8..0

# Per-core column layout: level-major (leaves first), and within each
# level the EVEN (left-sibling) nodes form one block followed by the ODD
# nodes — so every deinterleaving DVE op reads/writes contiguously.
# Roots (level 0) come last as one plain block.
EOFF = {}
_off = 0
for _d in range(DEPTH, 0, -1):
    _m = TPC * (1 << _d)
    EOFF[(_d, 0)] = _off
    EOFF[(_d, 1)] = _off + _m // 2
    _off += _m
EOFF[(0, 0)] = _off
_off += TPC
NCOL = _off                      # 8184
NLEAFC = TPC * (1 << DEPTH)      # 4096 leaf columns
NLEAFH = NLEAFC // 2             # leaf even/odd block size
NCOLI = NCOL - NLEAFC            # 4088 internal columns

COLTILE = 512
ROWR = 32           # partition base of the right-child (odd) block
NROWY = 52          # ybuf partitions: 0..19 left, 32..51 right (20..31 zero)
KCH = 4             # 400 = 4 x 100 chunks of the ij axis
CHW = IJ // KCH     # 100

# phase-1 DMA column groups: internal columns first (small leading tiles
# for an early start), leaves LAST so phase-2 starts only after phase-1 —
# a dense phase-2 block keeps PE duty high enough to hold the 2.4 GHz
# clock (interleaving with the DMA-paced phase-1 left PE at ~75% duty,
# which the HAM punishes with a 1.2 GHz clock for the whole kernel).
DMA_GROUPS = ((0, 512), (512, 512), (1024, 1024), (2048, 2048),
              (4096, 2048), (6144, 2040))
assert sum(dw for _, dw in DMA_GROUPS) == NCOL

# phase-2 column tile per level (splits enable cross-stage pipelining;
# below level 5 the two half-tree groups form independent chains that
# overlap each other's engine hops)
P2_TILE = {8: 512, 7: 512, 6: 256, 5: 128, 4: 64, 3: 32, 2: 16}


def _host_constants(W, b, trans):
    # expT: [400, 20], row = (lL*4+lc)*20 + (rL*4+rc), col = pL*4+pc,
    # chunked to [100, 4, 20] so SBUF tiles slice on a free dim.
    expT = np.exp(trans.astype(np.float64).transpose(1, 4, 2, 5, 0, 3)
                  .reshape(IJ, LC))
    expT_ch = np.ascontiguousarray(
        expT.reshape(KCH, CHW, LC).transpose(1, 0, 2)).astype(NP_BF16)

    ij = np.arange(IJ)
    # select-with-normalization: row i -> +1 where lL*C+lc == i, row 0 -> -1
    # everywhere (cancels to 0 for i == 0); same for the right block.
    selLR = np.zeros((NROWY, IJ), np.float32)
    selLR[ij // LC, ij] += 1.0           # left-child state select
    selLR[0, :] -= 1.0                   # minus left state-0 (normalize)
    selLR[ROWR + ij % LC, ij] += 1.0     # right-child state select
    selLR[ROWR, :] -= 1.0                # minus right state-0
    selLR = selLR.astype(NP_BF16)

    # stationary weights: plain W everywhere — every buffer row stores an
    # absolute-representation score (uniform per-node shift), so the select
    # matrices' -1 rows do all normalization.  fp8 + DoubleRow layout:
    # [p, kc, kt, LC] with W row r = kc*256 + kt*128 + p.
    w_sb = np.zeros((128, 2, 2, 32), NP_FP8)         # LC padded to 32 so
    w_sb[:, :, :, :LC] = W.reshape(2, 2, 128, LC).transpose(
        2, 0, 1, 3).astype(NP_FP8)                   # kt-stride is 16B-aligned
    bcols = b.astype(np.float32).reshape(LC, 1)

    ones_row = np.ones((1, LC), np.float32)
    return {
        "expt": expT_ch, "sellr": selLR, "wboth": w_sb, "bcols": bcols,
        "onesr": ones_row,
    }


def _host_ht(h, core):
    """fp8 [2, 128, 2, NCOL] slice for one core: level-major with
    even/odd node sub-blocks, t-major inside; row r = kc*256 + kt*128 + p
    (DoubleRow k-subtile layout)."""
    hk = h[core * TPC:(core + 1) * TPC]          # [8, 1023, 512]
    blocks = []
    for d in range(DEPTH, 0, -1):
        lo, hi = (1 << d) - 1, (1 << (d + 1)) - 1
        blk = hk[:, lo:hi, :].transpose(2, 0, 1).reshape(D, -1)
        blocks.append(blk[:, 0::2])
        blocks.append(blk[:, 1::2])
    blocks.append(hk[:, 0:1, :].transpose(2, 0, 1).reshape(D, -1))
    out = np.concatenate(blocks, axis=1)         # [512, NCOL]
    out = out.reshape(2, 2, 128, NCOL).transpose(0, 2, 1, 3)
    return np.ascontiguousarray(out).astype(NP_FP8)


def _patch_act_tables(nc):
    """Retarget every activation-table load to natural_log_exp_and_others
    (covers Exp, Ln and Identity) and drop the now-redundant reloads, which
    otherwise cost ~1.3us each when Exp and Ln alternate."""
    from concourse.hw_specs import get_activation_tables
    tables = list(get_activation_tables(nc.m.arch).items())
    target = None
    for idx, (name, _fns) in enumerate(tables):
        if name == "natural_log_exp_and_others":
            target = idx
    if target is None:
        return
    for fn in nc.m.functions:
        kept = False
        for blk in fn.blocks:
            new_insts = []
            for ins in blk.instructions:
                if isinstance(ins, mybir.InstLoadActFuncSet):
                    si = ins.sync_info
                    has_sems = si is not None and (
                        len(si.on_wait) > 0 or len(si.on_update) > 0)
                    if not kept or has_sems:
                        ins.act_func_set_id = target
                        kept = True
                        new_insts.append(ins)
                    continue
                new_insts.append(ins)
            blk.instructions[:] = new_insts


def _build_bass():
    nc = bacc.Bacc("TRN2", target_bir_lowering=False)

    ht_d = nc.declare_dram_parameter("ht", [2, 128, 2, NCOL], FP8,
                                     isOutput=False)
    wboth_d = nc.declare_dram_parameter("wboth", [128, 2, 2, 32],
                                        FP8, isOutput=False)
    bcols_d = nc.declare_dram_parameter("bcols", [LC, 1], F32, isOutput=False)
    expt_d = nc.declare_dram_parameter("expt", [CHW, KCH, LC], BF16,
                                       isOutput=False)
    sellr_d = nc.declare_dram_parameter("sellr", [NROWY, IJ], BF16,
                                        isOutput=False)
    onesr_d = nc.declare_dram_parameter("onesr", [1, LC], F32, isOutput=False)
    out_d = nc.declare_dram_parameter("out", [LC, TPC], F32, isOutput=True)

    EXP = mybir.ActivationFunctionType.Exp
    LN = mybir.ActivationFunctionType.Ln
    ADD = mybir.AluOpType.add

    with tile.TileContext(nc) as tc:
        with (
            tc.tile_pool(name="consts", bufs=1) as consts,
            tc.tile_pool(name="sw", bufs=1) as swp,
            tc.tile_pool(name="ybufs", bufs=1) as ybp,
            tc.tile_pool(name="ht", bufs=8) as htp,
            tc.tile_pool(name="vtiles", bufs=4) as vtp,
            tc.tile_pool(name="utiles", bufs=3) as utp,
            tc.tile_pool(name="ttiles", bufs=2) as ttp,
            tc.tile_pool(name="ps_sw", bufs=2, space="PSUM") as ps_swp,
            tc.tile_pool(name="ps_exp", bufs=2, space="PSUM") as ps_expp,
            tc.tile_pool(name="ps_out", bufs=2, space="PSUM") as ps_outp,
        ):
            # ---- constants: w_sb on the SP ring ahead of hT (it gates the
            # first matmul; the ACT ring is head-blocked by the act-table
            # load), everything else on the ACT HWDGE ring ----
            w_sb = consts.tile([128, 2, 2, 32], FP8)
            nc.sync.dma_start(w_sb[:], wboth_d[:])
            sellr_sb = consts.tile([NROWY, IJ], BF16)
            nc.scalar.dma_start(sellr_sb[:], sellr_d[:])
            expt_sb = consts.tile([CHW, KCH, LC], BF16)
            nc.scalar.dma_start(expt_sb[:], expt_d[:])
            b_sb = consts.tile([LC, 1], F32)
            nc.scalar.dma_start(b_sb[:], bcols_d[:])
            onesr_sb = consts.tile([1, LC], F32)
            nc.scalar.dma_start(onesr_sb[:], onesr_d[:])

            # absolute emissions of internal nodes
            sw_sb = swp.tile([LC, NCOLI], BF16)
            # per-tree accumulator of dropped carries (z-total)
            zacc = swp.tile([1, TPC], F32)
            nc.vector.memset(zacc[:], 0.0)

            # per-level deinterleaved buffers, bf16; rows 20:32 are dead
            # (zero select weight) but must be finite for the matmul
            ybufs = {}
            for d in range(DEPTH, 0, -1):
                npair = TPC * (1 << d) // 2
                yb = ybp.tile([NROWY, npair], BF16, tag=f"y{d}", name=f"y{d}")
                nc.gpsimd.memset(yb[0:ROWR, :], 0.0)
                ybufs[d] = yb

            # ---- phase 1: sw / leaf level from hT ----
            for ct, dw in DMA_GROUPS:
                htts = []
                for kc in range(2):
                    htt = htp.tile([128, 2, 2048], FP8, tag="htt",
                                   name="htt")
                    nc.sync.dma_start(htt[:, :, :dw],
                                      ht_d[kc, :, :, ct:ct + dw])
                    htts.append(htt)
                for half in range(0, dw, COLTILE):
                    nt = min(COLTILE, dw - half)
                    c0 = ct + half
                    leaf = c0 < NLEAFC
                    pool = ps_outp if leaf else ps_swp
                    tag = "ps_out" if leaf else "ps_sw"
                    ps = pool.tile([LC, COLTILE], F32, tag=tag, name="ps")
                    for kc in range(2):
                        nc.tensor.matmul(
                            ps[:, :nt], w_sb[:, kc, :, 0:LC],
                            htts[kc][:, :, half:half + nt],
                            start=(kc == 0), stop=(kc == 1),
                            perf_mode=mybir.MatmulPerfMode.DoubleRow)
                    if leaf:
                        # absolute leaf emissions (already deinterleaved);
                        # drained by ACT (idle until the first level-8 exp)
                        # via Identity + per-partition bias
                        y9 = ybufs[DEPTH]
                        if c0 < NLEAFH:
                            nc.scalar.activation(
                                y9[0:LC, c0:c0 + nt], ps[:, :nt],
                                mybir.ActivationFunctionType.Identity,
                                bias=b_sb[:, 0:1])
                        else:
                            nc.scalar.activation(
                                y9[ROWR:ROWR + LC, c0 - NLEAFH:
                                   c0 - NLEAFH + nt], ps[:, :nt],
                                mybir.ActivationFunctionType.Identity,
                                bias=b_sb[:, 0:1])
                    else:
                        nc.vector.tensor_scalar(
                            sw_sb[0:LC, c0 - NLEAFC:c0 - NLEAFC + nt],
                            ps[:, :nt], b_sb[:, 0:1], None, ADD)

            # ---- phase 2: bottom-up tree levels ----
            def _fold_z(yb_child):
                for row in (0, ROWR):
                    rsum = utp.tile([1, TPC], F32, tag="rsum", name="rsum")
                    nc.vector.tensor_reduce(
                        rsum[:], yb_child[row:row + 1, :].rearrange(
                            "p (t q) -> p t q", t=TPC),
                        mybir.AxisListType.X, ADD)
                    nc.vector.tensor_add(zacc[:], zacc[:], rsum[:])

            for d in range(DEPTH - 1, -1, -1):
                n = TPC * (1 << d)
                yprev = ybufs[d + 1]
                pe_off = EOFF[(d, 0)] - NLEAFC
                po_off = (EOFF[(d, 1)] - NLEAFC) if d > 0 else 0
                if d == 0:
                    _fold_z(yprev)
                ptile = P2_TILE.get(d, n)
                for c0 in range(0, n, ptile):
                    nt = min(ptile, n - c0)
                    ops_ = ps_outp.tile([LC, COLTILE], F32, tag="ps_out",
                                        name="ps_out")
                    if nt <= 256:
                        # all 4 ij-chunks in one PSUM tile; single ACT exp
                        eps = ps_expp.tile([CHW, 2, COLTILE], F32,
                                           tag="ps_exp", name="ps_exp")
                        for kc in range(KCH):
                            sl = eps[:, kc // 2,
                                     (kc % 2) * nt:(kc % 2) * nt + nt]
                            nc.tensor.matmul(
                                sl, sellr_sb[:, kc * CHW:(kc + 1) * CHW],
                                yprev[0:NROWY, c0:c0 + nt],
                                start=True, stop=True)
                        v_sb = vtp.tile([CHW, 2, COLTILE], BF16, tag="v",
                                        name="v")
                        nc.scalar.activation(v_sb[:, :, :2 * nt],
                                             eps[:, :, :2 * nt], EXP)
                        for kc in range(KCH):
                            vsl = v_sb[:, kc // 2,
                                       (kc % 2) * nt:(kc % 2) * nt + nt]
                            nc.tensor.matmul(
                                ops_[:, :nt], expt_sb[:, kc, :], vsl,
                                start=(kc == 0), stop=(kc == KCH - 1))
                    else:
                        for kp in range(KCH // 2):
                            eps = ps_expp.tile([CHW, 2, COLTILE], F32,
                                               tag="ps_exp", name="ps_exp")
                            for kk in range(2):
                                kc = 2 * kp + kk
                                nc.tensor.matmul(
                                    eps[:, kk, :nt],
                                    sellr_sb[:, kc * CHW:(kc + 1) * CHW],
                                    yprev[0:NROWY, c0:c0 + nt],
                                    start=True, stop=True)
                            v_sb = vtp.tile([CHW, 2, COLTILE], BF16, tag="v",
                                            name="v")
                            nc.scalar.activation(v_sb[:, :, :nt],
                                                 eps[:, :, :nt], EXP)
                            for kk in range(2):
                                kc = 2 * kp + kk
                                nc.tensor.matmul(
                                    ops_[:, :nt], expt_sb[:, kc, :],
                                    v_sb[:, kk, :nt],
                                    start=(kc == 0), stop=(kc == KCH - 1))

                    if d == 0:
                        t_sb = ttp.tile([LC, COLTILE], F32, tag="t0",
                                        name="t")
                        nc.scalar.activation(t_sb[:, :nt], ops_[:, :nt], LN)
                        # final: Y = t + p_abs + ztotal bcast
                        qps = ps_outp.tile([LC, COLTILE], F32,
                                           tag="ps_out", name="qps")
                        nc.tensor.matmul(qps[:, :nt], onesr_sb[:],
                                         zacc[:], start=True, stop=True)
                        y0a = utp.tile([LC, TPC], F32, tag="y0a", name="y0a")
                        nc.vector.tensor_add(
                            y0a[:], t_sb[:, :nt],
                            sw_sb[0:LC, pe_off:pe_off + nt])
                        y0b = utp.tile([LC, TPC], F32, tag="y0b", name="y0b")
                        nc.vector.tensor_add(y0b[:], y0a[:], qps[:, :nt])
                        nc.sync.dma_start(out_d[:], y0b[:])
                        continue

                    nh = nt // 2
                    t_sb = ttp.tile([LC, COLTILE], BF16, tag="t2",
                                    name="t")
                    nc.scalar.activation(t_sb[:, :nt], ops_[:, :nt], LN)

                    # u = t + p into the next-level buffer (sw contiguous)
                    pair0 = c0 // 2
                    yb = ybufs[d]
                    nc.vector.tensor_add(
                        yb[0:LC, pair0:pair0 + nh], t_sb[:, 0:nt:2],
                        sw_sb[0:LC, pe_off + pair0:pe_off + pair0 + nh])
                    nc.vector.tensor_add(
                        yb[ROWR:ROWR + LC, pair0:pair0 + nh],
                        t_sb[:, 1:nt:2],
                        sw_sb[0:LC, po_off + pair0:po_off + pair0 + nh])
                if d > 0:
                    _fold_z(yprev)

    nc.compile()
    _patch_act_tables(nc)
    return nc


_CACHE = {}


def _get_nc():
    if "nc" not in _CACHE:
        _CACHE["nc"] = _build_bass()
    return _CACHE["nc"]


def run(h, W, b, trans, trace=False, **trace_kwargs):
    h = np.asarray(h, dtype=np.float32)
    W = np.asarray(W, dtype=np.float32)
    b = np.asarray(b, dtype=np.float32)
    trans = np.asarray(trans, dtype=np.float32)

    consts = _host_constants(W, b, trans)
    in_maps = []
    for core in range(NCORES):
        m = dict(consts)
        m["ht"] = _host_ht(h, core)
        in_maps.append(m)

    nc = _get_nc()
    res = run_bass_kernel_spmd(nc, in_maps, list(range(NCORES)),
                               trace=trace, **trace_kwargs)
    outs = [res.results[k]["out"] for k in range(NCORES)]  # each [20, 8]
    full = np.concatenate([np.asarray(o, np.float32).T for o in outs],
                          axis=0).reshape(B, L, C)
    return np.ascontiguousarray(full), res


def kernel(h, W, b, trans):
    out, _ = run(h, W, b, trans, trace=False)
    return out
